# revision 38
# baseline (speedup 1.0000x reference)
"""Trainium2 Bass kernel for CognitionNetwork (GNN message passing + LSTM attention).

Contract: kernel(**inputs) takes FULL inputs, returns FULL [2048, 400] q_star.
Shards 2048 conversations across 8 NeuronCores (256 segments each, bin-packed);
each block of 32 segments owns TL[g] 128-node tiles (host re-layout).

v4 design (vs v3.1):
  - segments are bin-packed into blocks with a per-core tile profile TL
    (e.g. [17,16,16,16,16,16,16,16]) identical on every core, minimizing the
    total node-tile count NT and therefore PE streaming work; the host
    permutes segments into blocks and un-permutes the output.
  - attention e from block-level matmuls contracting FEATURES with the mask
    folded into 33 aug rows; exp straight out of PSUM into bf16.
  - EA node-major flip via one XBAR DMA-transpose per q-group (sync queue,
    manual RAW/WAR edges since DMA-transpose reads are not dep-tracked).
  - LSTM for step s+1 emitted inside step s's attention tails (per half);
    sigmoid computed as 0.5*tanh(x/2)+0.5 so the scalar engine never swaps
    activation tables (exp/tanh/copy share one set).
"""

import os
from contextlib import ExitStack

import ml_dtypes
import numpy as np

import concourse.bass as bass
import concourse.bacc as bacc
import concourse.tile as tile
from concourse.tile_rust import add_dep_helper
from concourse import mybir
from concourse.bass_utils import run_bass_kernel_spmd

CORES = 8
B = 2048
F = 200
FW = 201              # node-major x tile width: 200 feats + ones col
SEG_PER_CORE = B // CORES   # 256
BS = 32               # segments per block
BLOCKS = SEG_PER_CORE // BS  # 8
STEPS = 3
KAUG = F + BS + 1     # 233 feature rows incl mask aug
K2 = KAUG - 128       # 105 rows in chunk 2

TRACE = bool(int(os.environ.get("KERNEL_TRACE", "0")))
LAST_RESULT = None
_PROG_CACHE = {}

# LSTM weight chunk profiles (kdim per 128-row slab in the host-packed stacks)
W0_KD = [128, 128, 128, 17, 128, 105]   # q_star(400)+bias | h(200)+augzeros(33)
WC_KD = [128, 105, 128, 73]             # q(200)+aug | r(200)+bias
QS_KD = [128, 128, 128, 17]             # q_star0^T(400)+ones row


def _build_program(TL, nsteps: int = STEPS) -> bass.Bass:
    TL = list(TL)
    NT = sum(TL)                 # node tiles per core
    BO = [0]
    for t in TL:
        BO.append(BO[-1] + t)    # tile offset per block
    XFW = NT * 128               # feature-major x width (nodes)
    Tmax = max(TL)
    TQ = [max(TL[0:4]), max(TL[4:8])]   # per-quad max tiles

    nc = bacc.Bacc("TRN2", target_bir_lowering=False, debug=False)
    f32 = mybir.dt.float32
    f32r = mybir.dt.float32r
    f16 = mybir.dt.float16
    bf16 = mybir.dt.bfloat16
    AF = mybir.ActivationFunctionType
    ALU = mybir.AluOpType

    xf1_d = nc.dram_tensor("xf1", [128, XFW], f16, kind="ExternalInput").ap()
    xf2_d = nc.dram_tensor("xf2", [K2, XFW], f16, kind="ExternalInput").ap()
    cwt_d = nc.dram_tensor("cwt", [128, NT * BS], f16, kind="ExternalInput").ap()
    xp_d = nc.dram_tensor("xp", [128, NT * FW], f16, kind="ExternalInput").ap()
    qs0s_d = nc.dram_tensor("qs0s", [128, 4 * 256], f16, kind="ExternalInput").ap()
    w0s_d = nc.dram_tensor("w0s", [128, 6 * 800], f16, kind="ExternalInput").ap()
    wcs_d = nc.dram_tensor("wcs", [128, 4 * 800], f16, kind="ExternalInput").ap()
    qc2c_d = nc.dram_tensor("qc2c", [BS + 1, 256], f16, kind="ExternalInput").ap()
    ones_d = nc.dram_tensor("onesr", [1, 256], f16, kind="ExternalInput").ap()
    idf_d = nc.dram_tensor("idf", [128, 128], f32r, kind="ExternalInput").ap()
    qout_d = nc.dram_tensor("qout", [256, 400], f32, kind="ExternalOutput").ap()

    with tile.TileContext(nc) as tc:
        with ExitStack() as ctx:
            res = ctx.enter_context(tc.tile_pool(name="res", bufs=1))
            state = ctx.enter_context(tc.tile_pool(name="state", bufs=1))
            eap = ctx.enter_context(tc.tile_pool(name="eap", bufs=2))
            xpp = ctx.enter_context(tc.tile_pool(name="xpp", bufs=3))
            eanp = ctx.enter_context(tc.tile_pool(name="eanp", bufs=2))
            sbt = ctx.enter_context(tc.tile_pool(name="sbt", bufs=2))
            psE = ctx.enter_context(tc.tile_pool(name="psE", bufs=3, space="PSUM"))
            psG = ctx.enter_context(tc.tile_pool(name="psG", bufs=2, space="PSUM"))
            psT = ctx.enter_context(tc.tile_pool(name="psT", bufs=2, space="PSUM"))
            psR = ctx.enter_context(tc.tile_pool(name="psR", bufs=1, space="PSUM"))

            # ---------------- resident tiles ----------------
            idf = res.tile([128, 128], f32r)
            nc.scalar.dma_start(idf[:], idf_d[:])

            cwt_sb = res.tile([128, NT * BS], f16)
            xnm_sb = res.tile([128, NT * FW], bf16)
            xf1_sb = res.tile([128, XFW], f16)
            xf2_sb = res.tile([K2, XFW], f16)

            # transposed-input chunks: Q1/Q2 (h^T + mask const), R1/R2 (r^T + ones)
            Q1 = res.tile([128, 256], f16, tag="Q1", name="Q1")
            Q2 = res.tile([K2, 256], f16, tag="Q2", name="Q2")
            R1 = res.tile([128, 256], f16, tag="R1", name="R1")
            R2 = res.tile([73, 256], f16, tag="R2", name="R2")

            # fp32 state masters (seg-major, two 128-partition halves)
            h_sb = [state.tile([128, F], f32r, tag=f"h{i}", name=f"h{i}") for i in range(2)]
            c_sb = [state.tile([128, F], f32, tag=f"c{i}", name=f"c{i}") for i in range(2)]
            r_sb = [state.tile([128, F], f32r, tag=f"r{i}", name=f"r{i}") for i in range(2)]
            for i in range(2):
                nc.vector.memset(c_sb[i][:], 0.0)

            w0s = res.tile([128, 6 * 800], f16, tag="w0s", name="w0s")
            qs0s = res.tile([128, 4 * 256], f16, tag="qs0s", name="qs0s")

            # ---------------- phase 0: h0 = segment_sum(cos * x) ----------------
            # quad-stacked; streams fp16 x (with ones col) per block, casting it
            # into the resident bf16 node-major copy as it goes
            for q in range(2):
                h0ps = psR.tile([128, F], f32, tag="rblk")
                for a in range(4):
                    g = 4 * q + a
                    T = TL[g]
                    eng = nc.scalar if g % 2 else nc.sync
                    eng.dma_start(
                        cwt_sb[:, BO[g] * BS : BO[g + 1] * BS],
                        cwt_d[:, BO[g] * BS : BO[g + 1] * BS],
                    )
                    xpt = xpp.tile([128, Tmax * FW], f16, tag="xp")
                    XW = T * FW
                    XH = XW // 2
                    o0 = BO[g] * FW
                    nc.sync.dma_start(xpt[:, 0:XH], xp_d[:, o0 : o0 + XH])
                    nc.scalar.dma_start(xpt[:, XH:XW], xp_d[:, o0 + XH : o0 + XW])
                    for i in range(T):
                        nc.tensor.matmul(
                            h0ps[32 * a : 32 * a + 32, :],
                            lhsT=cwt_sb[:, (BO[g] + i) * BS : (BO[g] + i + 1) * BS],
                            rhs=xpt[:, i * FW : i * FW + F],
                            start=(i == 0),
                            stop=(i == T - 1),
                            tile_position=(0, 32 * a),
                        )
                    nc.vector.tensor_copy(xnm_sb[:, o0 : o0 + XW], xpt[:, 0:XW])
                nc.vector.tensor_copy(h_sb[q][:], h0ps[:])

            # remaining loads, in consumption order: LSTM0 weights + aug, then
            # feature-major x for attention, then step>=1 weights
            nc.sync.dma_start(w0s[:], w0s_d[:])
            nc.scalar.dma_start(qs0s[:], qs0s_d[:])
            nc.sync.dma_start(Q2[72:K2, :], qc2c_d[:])
            for g in range(BLOCKS):
                eng = nc.scalar if g < 4 else nc.sync
                c0, c1 = BO[g] * 128, BO[g + 1] * 128
                eng.dma_start(xf1_sb[:, c0:c1], xf1_d[:, c0:c1])
                eng.dma_start(xf2_sb[:, c0:c1], xf2_d[:, c0:c1])
            wcs = res.tile([128, 4 * 800], f16, tag="wcs", name="wcs")
            nc.scalar.dma_start(wcs[:], wcs_d[:])
            nc.sync.dma_start(R2[72:73, :], ones_d[:])

            def emit_hT(src_halves, dst1, dst2, halves=(0, 1)):
                """transpose seg-major [128,200] f32r halves into fp16 feat-major
                chunks: dst1[:, co:co+128] rows 0..127, dst2[0:72, ...] rows 128..199."""
                for half in halves:
                    src = src_halves[half]
                    co = 128 * half
                    t1 = psT.tile([128, 128], f32r, tag="tp")
                    nc.tensor.transpose(t1[:], src[:, 0:128], idf[:])
                    nc.vector.tensor_copy(dst1[:, co : co + 128], t1[:].bitcast(f32))
                    t2 = psT.tile([72, 128], f32r, tag="tp")
                    nc.tensor.transpose(t2[:], src[:, 128:200], idf[:])
                    nc.vector.tensor_copy(dst2[0:72, co : co + 128], t2[:].bitcast(f32))

            emit_hT(h_sb, Q1, Q2)

            # ---------------- LSTM cell ----------------
            def lstm_half(half, step0):
                co = 128 * half
                if step0:
                    chunks = [(qs0s, ci * 256, kd) for ci, kd in enumerate(QS_KD)]
                    chunks += [(Q1, None, 128), (Q2, None, K2)]
                    wts = w0s
                else:
                    chunks = [(Q1, None, 128), (Q2, None, K2),
                              (R1, None, 128), (R2, None, 73)]
                    wts = wcs
                # sigmoid(x) = 0.5*tanh(x/2) + 0.5: keeps every scalar-engine
                # activation (exp/tanh/copy) inside one act-table set -> no
                # ACT_TABLE_LOAD swaps on the critical path.
                acts = {}
                for part in range(2):
                    ps = psG.tile([128, 400], f32, tag="gates")
                    nch = len(chunks)
                    for ci, (ctile, coff, kdim) in enumerate(chunks):
                        lhsT = (ctile[0:kdim, coff + co : coff + co + 128]
                                if coff is not None
                                else ctile[0:kdim, co : co + 128])
                        nc.tensor.matmul(
                            ps[:],
                            lhsT=lhsT,
                            rhs=wts[0:kdim, ci * 800 + 400 * part : ci * 800 + 400 * part + 400],
                            start=(ci == 0),
                            stop=(ci == nch - 1),
                        )
                    if part == 0:
                        # both gates take scale=0.5: one batched activation
                        tif = sbt.tile([128, 400], f32, tag="tif")
                        nc.scalar.activation(tif[:], ps[:], AF.Tanh, scale=0.5)
                        nc.vector.tensor_scalar(tif[:], tif[:], 0.5, 0.5, ALU.mult, ALU.add)
                        acts["i"], acts["f"] = tif[:, 0:F], tif[:, F:400]
                    else:
                        tg = sbt.tile([128, F], f32, tag="tg")
                        nc.scalar.activation(tg[:], ps[:, 0:F], AF.Tanh)
                        to = sbt.tile([128, F], f32, tag="so")
                        nc.scalar.activation(to[:], ps[:, F:400], AF.Tanh, scale=0.5)
                        nc.vector.tensor_scalar(to[:], to[:], 0.5, 0.5, ALU.mult, ALU.add)
                        acts["g"], acts["o"] = tg[:], to[:]
                ch = c_sb[half]
                tmp = sbt.tile([128, F], f32, tag="tmp")
                nc.vector.tensor_mul(tmp[:], acts["f"], ch[:])
                nc.vector.tensor_mul(ch[:], acts["i"], acts["g"])
                nc.vector.tensor_add(ch[:], tmp[:], ch[:])
                tct = sbt.tile([128, F], f32, tag="tct")
                nc.scalar.activation(tct[:], ch[:], AF.Tanh)
                nc.vector.tensor_mul(h_sb[half][:], acts["o"], tct[:])

            # first LSTM step (h0 + given q_star)
            if nsteps >= 1:
                lstm_half(0, True)
                lstm_half(1, True)
                if nsteps == 1:
                    for half in range(2):
                        nc.sync.dma_start(
                            qout_d[128 * half : 128 * half + 128, 0:F],
                            h_sb[half][:].bitcast(f32),
                        )

            # ---------------- attention ----------------
            prev_dmat = [None, None]

            def emit_e(q):
                """e_aug matmuls + exp for 4 stacked blocks -> EA [128, TQ[q]*128] bf16."""
                BWq = TQ[q] * 128
                ea = eap.tile([128, Tmax * 128], bf16, tag="ea", name="ea")
                exps = []
                NCH = (BWq + 511) // 512
                for k in range(NCH):
                    c0 = k * 512
                    cwm = min(512, BWq - c0)
                    pe = psE.tile([128, 512], f32, tag="pe")
                    for a in range(4):
                        g = 4 * q + a
                        cw = min(512, TL[g] * 128 - c0)
                        if cw <= 0:
                            continue
                        nb = BO[g] * 128
                        nc.tensor.matmul(
                            pe[32 * a : 32 * a + 32, 0:cw],
                            lhsT=Q1[:, BS * g : BS * (g + 1)],
                            rhs=xf1_sb[:, nb + c0 : nb + c0 + cw],
                            start=True,
                            stop=False,
                            tile_position=(0, 32 * a),
                        )
                        nc.tensor.matmul(
                            pe[32 * a : 32 * a + 32, 0:cw],
                            lhsT=Q2[0:K2, BS * g : BS * (g + 1)],
                            rhs=xf2_sb[0:K2, nb + c0 : nb + c0 + cw],
                            start=False,
                            stop=True,
                            tile_position=(0, 32 * a),
                        )
                    ei = nc.scalar.activation(ea[:, c0 : c0 + cwm], pe[:, 0:cwm], AF.Exp)
                    exps.append(ei)
                if prev_dmat[q] is not None:
                    # the DMA-transpose READ of ea is not dependency-tracked:
                    # order this buffer's first overwrite after the previous
                    # step's transpose explicitly (WAR).
                    add_dep_helper(exps[0].ins, prev_dmat[q].ins,
                                   reason="ea WAR vs untracked dma-transpose read")
                return ea, exps[-1]

            def emit_eanT(q, ea, last_exp):
                """XBAR dma-transpose EA node-major on the sync queue. The
                transpose's READ of ea is not dependency-tracked, so add the
                RAW edge on the last exp writer manually."""
                ean = eanp.tile([128, Tmax * 128], bf16, tag="ean")
                dmat = nc.sync.dma_start(
                    ean[:, 0 : TQ[q] * 128].rearrange("p (t c) -> p t c", t=TQ[q]),
                    ea[:, 0 : TQ[q] * 128],
                    transpose=True,
                )
                add_dep_helper(dmat.ins, last_exp.ins,
                               reason="dma-transpose untracked read of ea (RAW)")
                prev_dmat[q] = dmat
                return ean

            def emit_attn_tail(q, ean):
                """r matmuls over the node-major attention, then normalize."""
                rps = psR.tile([128, F + 1], f32, tag="rblk")
                for i in range(TQ[q]):
                    for a in range(4):
                        g = 4 * q + a
                        if i >= TL[g]:
                            continue
                        t = BO[g] + i
                        nc.tensor.matmul(
                            rps[32 * a : 32 * a + 32, :],
                            lhsT=ean[:, 128 * i + 32 * a : 128 * i + 32 * a + 32],
                            rhs=xnm_sb[:, t * FW : t * FW + F + 1],
                            start=(i == 0),
                            stop=(i == TL[g] - 1),
                            tile_position=(0, 32 * a),
                        )
                dinv = sbt.tile([128, 1], f32, tag="dinv")
                nc.vector.reciprocal(dinv[:], rps[:, F : F + 1])
                nc.vector.tensor_scalar(r_sb[q][:], rps[:, 0:F], dinv[:], None, ALU.mult)

            # ---------------- steps ----------------
            for s in range(nsteps):
                emit_hT(h_sb, Q1, Q2, halves=(0,))
                ea0, le0 = emit_e(0)
                ean0 = emit_eanT(0, ea0, le0)
                emit_hT(h_sb, Q1, Q2, halves=(1,))
                ea1, le1 = emit_e(1)
                ean1 = emit_eanT(1, ea1, le1)

                emit_attn_tail(0, ean0)
                if s < nsteps - 1:
                    # next LSTM step, half 0: runs while half-1 attention streams
                    emit_hT(r_sb, R1, R2, halves=(0,))
                    lstm_half(0, False)
                else:
                    nc.sync.dma_start(qout_d[0:128, F : 2 * F], r_sb[0][:].bitcast(f32))

                emit_attn_tail(1, ean1)
                if s < nsteps - 1:
                    emit_hT(r_sb, R1, R2, halves=(1,))
                    lstm_half(1, False)
                else:
                    nc.sync.dma_start(qout_d[128:256, F : 2 * F], r_sb[1][:].bitcast(f32))
                if s == nsteps - 2:
                    # that was the final LSTM: h is the output q
                    for half in range(2):
                        nc.sync.dma_start(
                            qout_d[128 * half : 128 * half + 128, 0:F],
                            h_sb[half][:].bitcast(f32),
                        )

            if nsteps == 0:
                for half in range(2):
                    nc.sync.dma_start(
                        qout_d[128 * half : 128 * half + 128, 0:F], h_sb[half][:].bitcast(f32)
                    )

    nc.compile()
    return nc


def _get_program(TL) -> bass.Bass:
    nsteps = int(os.environ.get("KERNEL_NSTEPS", str(STEPS)))
    key = (tuple(TL), nsteps)
    if key not in _PROG_CACHE:
        _PROG_CACHE[key] = _build_program(TL, nsteps)
    return _PROG_CACHE[key]


def _pack_rows(src, kds, offs, width):
    """Stack row-chunks of src into [128, len(kds)*width] fp16."""
    out = np.zeros((128, len(kds) * width), np.float16)
    for ci, (kd, off) in enumerate(zip(kds, offs)):
        out[0:kd, ci * width : (ci + 1) * width] = src[off : off + kd]
    return out


def _bin_pack(counts, TL):
    """Pack 2048 segments into 64 bins (8 cores x 8 blocks), exactly 32 segs
    per bin, bin g (within core) holding <= TL[g]*128 nodes. Same TL profile
    per core. Returns list of 64 segment-id lists, or None on failure."""
    nbins = CORES * BLOCKS
    caps = np.array([TL[g] * 128 for _ in range(CORES) for g in range(BLOCKS)],
                    dtype=np.int64)
    rem = caps.copy()
    slots = np.full(nbins, BS, dtype=np.int64)
    bins = [[] for _ in range(nbins)]
    order = np.argsort(-counts, kind="stable")
    for s in order:
        avail = np.where(slots > 0)[0]
        b = avail[np.argmax(rem[avail])]
        if rem[b] < counts[s]:
            return None
        bins[b].append(int(s))
        rem[b] -= counts[s]
        slots[b] -= 1
    return bins


def make_in_maps(x, batch, cos_coef, q_star, W_ih, W_hh, b_ih, b_hh):
    """Host-side shard + re-layout. Returns (in_maps, TL, bins)."""
    x = np.ascontiguousarray(np.asarray(x, dtype=np.float32))
    batch = np.asarray(batch).astype(np.int64)
    cos = np.asarray(cos_coef, dtype=np.float32)
    qs = np.asarray(q_star, dtype=np.float32)
    W_ih = np.asarray(W_ih, dtype=np.float32)
    W_hh = np.asarray(W_hh, dtype=np.float32)
    bsum = (np.asarray(b_ih, dtype=np.float32) + np.asarray(b_hh, dtype=np.float32))

    counts = np.bincount(batch, minlength=B)
    starts = np.zeros(B + 1, dtype=np.int64)
    starts[1:] = np.cumsum(counts)

    base = int(max(1, -(-counts.reshape(-1, BS).sum(axis=1).max() // 128)))
    lo = base - 1
    profiles = [
        (base,) + (lo,) * 7,
        (base, base) + (lo,) * 6,
        (base,) * 4 + (lo,) * 4,
        (base,) * 8,
        (base + 1,) * 8,
    ]
    bins = None
    for TL in profiles:
        if min(TL) < 1:
            continue
        bins = _bin_pack(counts, TL)
        if bins is not None:
            break
    assert bins is not None
    TL = list(TL)
    NT = sum(TL)
    BO = [0]
    for t in TL:
        BO.append(BO[-1] + t)

    # LSTM weight stacks (fp16), pre-chunked to 128-row slabs
    W_ihT = W_ih.T  # [400, 800]
    W_hhT = W_hh.T  # [200, 800]
    w0 = np.concatenate(
        [W_ihT, bsum[None, :], W_hhT, np.zeros((BS + 1, 800), np.float32)], axis=0
    )  # [634, 800]
    w0s = _pack_rows(w0, W0_KD, [0, 128, 256, 384, 401, 529], 800)
    WcT = W_ihT[:F] + W_hhT          # [200, 800]
    WrT = W_ihT[F:]                  # [200, 800]
    wc = np.concatenate(
        [WcT[0:128], WcT[128:200], np.zeros((BS + 1, 800), np.float32),
         WrT[0:128], WrT[128:200], bsum[None, :]], axis=0
    )  # [434, 800]
    wcs = _pack_rows(wc, WC_KD, [0, 128, 233, 361], 800)

    qc2c = np.zeros((BS + 1, 256), np.float16)
    qc2c[0:BS] = np.tile(100.0 * np.eye(BS, dtype=np.float32), (1, BLOCKS))
    qc2c[BS] = -100.0

    in_maps = []
    for c in range(CORES):
        xf = np.zeros((KAUG, NT * 128), dtype=np.float16)
        cwt = np.zeros((128, NT * BS), dtype=np.float16)
        xp = np.zeros((128, NT * FW), dtype=np.float16)
        qs0t = np.ones((401, 256), dtype=np.float32)
        for g in range(BLOCKS):
            T = TL[g]
            BW = T * 128
            segs = bins[c * BLOCKS + g]
            idx = np.concatenate(
                [np.arange(starts[s], starts[s + 1]) for s in segs]
            )
            cnt = len(idx)
            js = np.concatenate(
                [np.full(int(counts[s]), j, np.int64) for j, s in enumerate(segs)]
            )

            xb = np.zeros((BW, FW), dtype=np.float32)
            xb[:cnt, :F] = x[idx]
            xb[:cnt, F] = 1.0
            xp[:, BO[g] * FW : BO[g + 1] * FW] = (
                xb.reshape(T, 128, FW).transpose(1, 0, 2).reshape(128, T * FW)
            ).astype(np.float16)

            xfb = np.zeros((KAUG, BW), dtype=np.float32)
            xfb[0:F, :cnt] = x[idx].T
            xfb[F + js, np.arange(cnt)] = 1.0
            xfb[F + BS, :] = 1.0
            xf[:, BO[g] * 128 : BO[g + 1] * 128] = xfb.astype(np.float16)

            wb = np.zeros((BW, BS), dtype=np.float32)
            wb[np.arange(cnt), js] = cos[idx]
            cwt[:, BO[g] * BS : BO[g + 1] * BS] = (
                wb.reshape(T, 128, BS).transpose(1, 0, 2).reshape(128, T * BS)
            ).astype(np.float16)

            qs0t[0:400, g * BS : (g + 1) * BS] = qs[segs].T
        qs0s = _pack_rows(qs0t, QS_KD, [0, 128, 256, 384], 256)
        in_maps.append(
            {
                "xf1": np.ascontiguousarray(xf[0:128]),
                "xf2": np.ascontiguousarray(xf[128:KAUG]),
                "cwt": cwt,
                "xp": xp,
                "qs0s": qs0s,
                "w0s": w0s,
                "wcs": wcs,
                "qc2c": qc2c,
                "onesr": np.ones((1, 256), np.float16),
                "idf": np.eye(128, dtype=np.float32),
            }
        )
    return in_maps, TL, bins


def kernel(x, batch, cos_coef, q_star, W_ih, W_hh, b_ih, b_hh):
    global LAST_RESULT
    in_maps, TL, bins = make_in_maps(
        x, batch, cos_coef, q_star, W_ih, W_hh, b_ih, b_hh
    )
    nc = _get_program(TL)
    res = run_bass_kernel_spmd(nc, in_maps, list(range(CORES)), trace=TRACE)
    LAST_RESULT = res
    out = np.zeros((B, 2 * F), dtype=np.float32)
    for c in range(CORES):
        qo = res.results[c]["qout"]
        for g in range(BLOCKS):
            segs = bins[c * BLOCKS + g]
            out[segs] = qo[g * BS : (g + 1) * BS]
    return out


# revision 39
# speedup vs baseline: 1.0405x; 1.0405x over previous
"""Trainium2 Bass kernel for CognitionNetwork (GNN message passing + LSTM attention).

Contract: kernel(**inputs) takes FULL inputs, returns FULL [2048, 400] q_star.
Shards 2048 conversations across 8 NeuronCores (256 segments each, bin-packed);
each block of 32 segments owns TL[g] 128-node tiles (host re-layout).

v4 design (vs v3.1):
  - segments are bin-packed into blocks with a per-core tile profile TL
    (e.g. [17,16,16,16,16,16,16,16]) identical on every core, minimizing the
    total node-tile count NT and therefore PE streaming work; the host
    permutes segments into blocks and un-permutes the output.
  - attention e from block-level matmuls contracting FEATURES with the mask
    folded into 33 aug rows; exp straight out of PSUM into bf16.
  - EA node-major flip via one XBAR DMA-transpose per q-group (sync queue,
    manual RAW/WAR edges since DMA-transpose reads are not dep-tracked).
  - LSTM for step s+1 emitted inside step s's attention tails (per half);
    sigmoid computed as 0.5*tanh(x/2)+0.5 so the scalar engine never swaps
    activation tables (exp/tanh/copy share one set).
"""

import os
from contextlib import ExitStack

import ml_dtypes
import numpy as np

import concourse.bass as bass
import concourse.bacc as bacc
import concourse.tile as tile
from concourse.tile_rust import add_dep_helper
from concourse import mybir
from concourse.bass_utils import run_bass_kernel_spmd

CORES = 8
B = 2048
F = 200
FW = 201              # node-major x tile width: 200 feats + ones col
SEG_PER_CORE = B // CORES   # 256
BS = 32               # segments per block
BLOCKS = SEG_PER_CORE // BS  # 8
STEPS = 3
KAUG = F + BS + 1     # 233 feature rows incl mask aug
K2 = KAUG - 128       # 105 rows in chunk 2

TRACE = bool(int(os.environ.get("KERNEL_TRACE", "0")))
LAST_RESULT = None
_PROG_CACHE = {}

# LSTM weight chunk profiles (kdim per 128-row slab in the host-packed stacks)
W0_KD = [128, 128, 128, 17, 128, 105]   # q_star(400)+bias | h(200)+augzeros(33)
WC_KD = [128, 105, 128, 73]             # q(200)+aug | r(200)+bias
QS_KD = [128, 128, 128, 17]             # q_star0^T(400)+ones row


def _build_program(TL, nsteps: int = STEPS) -> bass.Bass:
    TL = list(TL)
    NT = sum(TL)                 # node tiles per core
    BO = [0]
    for t in TL:
        BO.append(BO[-1] + t)    # tile offset per block
    XFW = NT * 128               # feature-major x width (nodes)
    Tmax = max(TL)
    TQ = [max(TL[0:4]), max(TL[4:8])]   # per-quad max tiles

    nc = bacc.Bacc("TRN2", target_bir_lowering=False, debug=False)
    f32 = mybir.dt.float32
    f32r = mybir.dt.float32r
    f16 = mybir.dt.float16
    bf16 = mybir.dt.bfloat16
    AF = mybir.ActivationFunctionType
    ALU = mybir.AluOpType

    xf1_d = nc.dram_tensor("xf1", [128, XFW], f16, kind="ExternalInput").ap()
    xf2_d = nc.dram_tensor("xf2", [K2, XFW], f16, kind="ExternalInput").ap()
    cwt_d = nc.dram_tensor("cwt", [128, NT * BS], f16, kind="ExternalInput").ap()
    xp_d = nc.dram_tensor("xp", [128, NT * FW], f16, kind="ExternalInput").ap()
    qs0s_d = nc.dram_tensor("qs0s", [128, 4 * 256], f16, kind="ExternalInput").ap()
    w0s_d = nc.dram_tensor("w0s", [128, 6 * 800], f16, kind="ExternalInput").ap()
    wcs_d = nc.dram_tensor("wcs", [128, 4 * 800], f16, kind="ExternalInput").ap()
    qc2c_d = nc.dram_tensor("qc2c", [BS + 1, 256], f16, kind="ExternalInput").ap()
    ones_d = nc.dram_tensor("onesr", [1, 256], f16, kind="ExternalInput").ap()
    idf_d = nc.dram_tensor("idf", [128, 128], f32r, kind="ExternalInput").ap()
    qout_d = nc.dram_tensor("qout", [256, 400], f32, kind="ExternalOutput").ap()

    with tile.TileContext(nc) as tc:
        with ExitStack() as ctx:
            res = ctx.enter_context(tc.tile_pool(name="res", bufs=1))
            state = ctx.enter_context(tc.tile_pool(name="state", bufs=1))
            eap = ctx.enter_context(tc.tile_pool(name="eap", bufs=2))
            xpp = ctx.enter_context(tc.tile_pool(name="xpp", bufs=4))
            eanp = ctx.enter_context(tc.tile_pool(name="eanp", bufs=2))
            sbt = ctx.enter_context(tc.tile_pool(name="sbt", bufs=2))
            psE = ctx.enter_context(tc.tile_pool(name="psE", bufs=3, space="PSUM"))
            psG = ctx.enter_context(tc.tile_pool(name="psG", bufs=2, space="PSUM"))
            psT = ctx.enter_context(tc.tile_pool(name="psT", bufs=2, space="PSUM"))
            psR = ctx.enter_context(tc.tile_pool(name="psR", bufs=1, space="PSUM"))

            # ---------------- resident tiles ----------------
            idf = res.tile([128, 128], f32r)
            nc.scalar.dma_start(idf[:], idf_d[:])

            cwt_sb = res.tile([128, NT * BS], f16)
            xnm_sb = res.tile([128, NT * FW], bf16)
            xf1_sb = res.tile([128, XFW], f16)
            xf2_sb = res.tile([K2, XFW], f16)

            # transposed-input chunks: Q1/Q2 (h^T + mask const), R1/R2 (r^T + ones)
            Q1 = res.tile([128, 256], f16, tag="Q1", name="Q1")
            Q2 = res.tile([K2, 256], f16, tag="Q2", name="Q2")
            R1 = res.tile([128, 256], f16, tag="R1", name="R1")
            R2 = res.tile([73, 256], f16, tag="R2", name="R2")

            # fp32 state masters (seg-major, two 128-partition halves)
            h_sb = [state.tile([128, F], f32r, tag=f"h{i}", name=f"h{i}") for i in range(2)]
            c_sb = [state.tile([128, F], f32, tag=f"c{i}", name=f"c{i}") for i in range(2)]
            r_sb = [state.tile([128, F], f32r, tag=f"r{i}", name=f"r{i}") for i in range(2)]
            for i in range(2):
                nc.vector.memset(c_sb[i][:], 0.0)

            w0s = res.tile([128, 6 * 800], f16, tag="w0s", name="w0s")
            qs0s = res.tile([128, 4 * 256], f16, tag="qs0s", name="qs0s")

            # ---------------- phase 0: h0 = segment_sum(cos * x) ----------------
            # quad-stacked; streams fp16 x (with ones col) per block, casting it
            # into the resident bf16 node-major copy as it goes
            for q in range(2):
                h0ps = psR.tile([128, F], f32, tag="rblk")
                for a in range(4):
                    g = 4 * q + a
                    T = TL[g]
                    eng = nc.scalar if g % 2 else nc.sync
                    eng.dma_start(
                        cwt_sb[:, BO[g] * BS : BO[g + 1] * BS],
                        cwt_d[:, BO[g] * BS : BO[g + 1] * BS],
                    )
                    xpt = xpp.tile([128, Tmax * FW], f16, tag="xp")
                    XW = T * FW
                    XH = XW // 2
                    o0 = BO[g] * FW
                    nc.sync.dma_start(xpt[:, 0:XH], xp_d[:, o0 : o0 + XH])
                    nc.scalar.dma_start(xpt[:, XH:XW], xp_d[:, o0 + XH : o0 + XW])
                    for i in range(T):
                        nc.tensor.matmul(
                            h0ps[32 * a : 32 * a + 32, :],
                            lhsT=cwt_sb[:, (BO[g] + i) * BS : (BO[g] + i + 1) * BS],
                            rhs=xpt[:, i * FW : i * FW + F],
                            start=(i == 0),
                            stop=(i == T - 1),
                            tile_position=(0, 32 * a),
                        )
                    nc.vector.tensor_copy(xnm_sb[:, o0 : o0 + XW], xpt[:, 0:XW])
                nc.vector.tensor_copy(h_sb[q][:], h0ps[:])

            # remaining loads, in consumption order: LSTM0 weights + aug, then
            # feature-major x for attention, then step>=1 weights
            nc.sync.dma_start(w0s[:], w0s_d[:])
            nc.scalar.dma_start(qs0s[:], qs0s_d[:])
            nc.sync.dma_start(Q2[72:K2, :], qc2c_d[:])
            for g in range(BLOCKS):
                eng = nc.scalar if g < 4 else nc.sync
                c0, c1 = BO[g] * 128, BO[g + 1] * 128
                eng.dma_start(xf1_sb[:, c0:c1], xf1_d[:, c0:c1])
                eng.dma_start(xf2_sb[:, c0:c1], xf2_d[:, c0:c1])
            wcs = res.tile([128, 4 * 800], f16, tag="wcs", name="wcs")
            nc.scalar.dma_start(wcs[:], wcs_d[:])
            nc.sync.dma_start(R2[72:73, :], ones_d[:])

            def emit_hT(src_halves, dst1, dst2, halves=(0, 1)):
                """transpose seg-major [128,200] f32r halves into fp16 feat-major
                chunks: dst1[:, co:co+128] rows 0..127, dst2[0:72, ...] rows 128..199."""
                for half in halves:
                    src = src_halves[half]
                    co = 128 * half
                    t1 = psT.tile([128, 128], f32r, tag="tp")
                    nc.tensor.transpose(t1[:], src[:, 0:128], idf[:])
                    nc.vector.tensor_copy(dst1[:, co : co + 128], t1[:].bitcast(f32))
                    t2 = psT.tile([72, 128], f32r, tag="tp")
                    nc.tensor.transpose(t2[:], src[:, 128:200], idf[:])
                    nc.vector.tensor_copy(dst2[0:72, co : co + 128], t2[:].bitcast(f32))

            emit_hT(h_sb, Q1, Q2)

            # ---------------- LSTM cell ----------------
            def lstm_half(half, step0):
                co = 128 * half
                if step0:
                    chunks = [(qs0s, ci * 256, kd) for ci, kd in enumerate(QS_KD)]
                    chunks += [(Q1, None, 128), (Q2, None, K2)]
                    wts = w0s
                else:
                    chunks = [(Q1, None, 128), (Q2, None, K2),
                              (R1, None, 128), (R2, None, 73)]
                    wts = wcs
                # sigmoid(x) = 0.5*tanh(x/2) + 0.5: keeps every scalar-engine
                # activation (exp/tanh/copy) inside one act-table set -> no
                # ACT_TABLE_LOAD swaps on the critical path.
                acts = {}
                for part in range(2):
                    ps = psG.tile([128, 400], f32, tag="gates")
                    nch = len(chunks)
                    for ci, (ctile, coff, kdim) in enumerate(chunks):
                        lhsT = (ctile[0:kdim, coff + co : coff + co + 128]
                                if coff is not None
                                else ctile[0:kdim, co : co + 128])
                        nc.tensor.matmul(
                            ps[:],
                            lhsT=lhsT,
                            rhs=wts[0:kdim, ci * 800 + 400 * part : ci * 800 + 400 * part + 400],
                            start=(ci == 0),
                            stop=(ci == nch - 1),
                        )
                    if part == 0:
                        # both gates take scale=0.5: one batched activation
                        tif = sbt.tile([128, 400], f32, tag="tif")
                        nc.scalar.activation(tif[:], ps[:], AF.Tanh, scale=0.5)
                        nc.vector.tensor_scalar(tif[:], tif[:], 0.5, 0.5, ALU.mult, ALU.add)
                        acts["i"], acts["f"] = tif[:, 0:F], tif[:, F:400]
                    else:
                        tg = sbt.tile([128, F], f32, tag="tg")
                        nc.scalar.activation(tg[:], ps[:, 0:F], AF.Tanh)
                        to = sbt.tile([128, F], f32, tag="so")
                        nc.scalar.activation(to[:], ps[:, F:400], AF.Tanh, scale=0.5)
                        nc.vector.tensor_scalar(to[:], to[:], 0.5, 0.5, ALU.mult, ALU.add)
                        acts["g"], acts["o"] = tg[:], to[:]
                ch = c_sb[half]
                tmp = sbt.tile([128, F], f32, tag="tmp")
                nc.vector.tensor_mul(tmp[:], acts["f"], ch[:])
                nc.vector.tensor_mul(ch[:], acts["i"], acts["g"])
                nc.vector.tensor_add(ch[:], tmp[:], ch[:])
                tct = sbt.tile([128, F], f32, tag="tct")
                nc.scalar.activation(tct[:], ch[:], AF.Tanh)
                nc.vector.tensor_mul(h_sb[half][:], acts["o"], tct[:])

            # first LSTM step (h0 + given q_star)
            if nsteps >= 1:
                lstm_half(0, True)
                lstm_half(1, True)
                if nsteps == 1:
                    for half in range(2):
                        nc.sync.dma_start(
                            qout_d[128 * half : 128 * half + 128, 0:F],
                            h_sb[half][:].bitcast(f32),
                        )

            # ---------------- attention ----------------
            prev_dmat = [None, None]

            def emit_e(q):
                """e_aug matmuls + exp for 4 stacked blocks -> EA [128, TQ[q]*128] bf16."""
                BWq = TQ[q] * 128
                ea = eap.tile([128, Tmax * 128], bf16, tag="ea", name="ea")
                exps = []
                NCH = (BWq + 511) // 512
                for k in range(NCH):
                    c0 = k * 512
                    cwm = min(512, BWq - c0)
                    pe = psE.tile([128, 512], f32, tag="pe")
                    for a in range(4):
                        g = 4 * q + a
                        cw = min(512, TL[g] * 128 - c0)
                        if cw <= 0:
                            continue
                        nb = BO[g] * 128
                        nc.tensor.matmul(
                            pe[32 * a : 32 * a + 32, 0:cw],
                            lhsT=Q1[:, BS * g : BS * (g + 1)],
                            rhs=xf1_sb[:, nb + c0 : nb + c0 + cw],
                            start=True,
                            stop=False,
                            tile_position=(0, 32 * a),
                        )
                        nc.tensor.matmul(
                            pe[32 * a : 32 * a + 32, 0:cw],
                            lhsT=Q2[0:K2, BS * g : BS * (g + 1)],
                            rhs=xf2_sb[0:K2, nb + c0 : nb + c0 + cw],
                            start=False,
                            stop=True,
                            tile_position=(0, 32 * a),
                        )
                    ei = nc.scalar.activation(ea[:, c0 : c0 + cwm], pe[:, 0:cwm], AF.Exp)
                    exps.append(ei)
                if prev_dmat[q] is not None:
                    # the DMA-transpose READ of ea is not dependency-tracked:
                    # order this buffer's first overwrite after the previous
                    # step's transpose explicitly (WAR).
                    add_dep_helper(exps[0].ins, prev_dmat[q].ins,
                                   reason="ea WAR vs untracked dma-transpose read")
                return ea, exps[-1]

            def emit_eanT(q, ea, last_exp):
                """XBAR dma-transpose EA node-major on the sync queue. The
                transpose's READ of ea is not dependency-tracked, so add the
                RAW edge on the last exp writer manually."""
                ean = eanp.tile([128, Tmax * 128], bf16, tag="ean")
                dmat = nc.sync.dma_start(
                    ean[:, 0 : TQ[q] * 128].rearrange("p (t c) -> p t c", t=TQ[q]),
                    ea[:, 0 : TQ[q] * 128],
                    transpose=True,
                )
                add_dep_helper(dmat.ins, last_exp.ins,
                               reason="dma-transpose untracked read of ea (RAW)")
                prev_dmat[q] = dmat
                return ean

            def emit_attn_tail(q, ean):
                """r matmuls over the node-major attention, then normalize."""
                rps = psR.tile([128, F + 1], f32, tag="rblk")
                for i in range(TQ[q]):
                    for a in range(4):
                        g = 4 * q + a
                        if i >= TL[g]:
                            continue
                        t = BO[g] + i
                        nc.tensor.matmul(
                            rps[32 * a : 32 * a + 32, :],
                            lhsT=ean[:, 128 * i + 32 * a : 128 * i + 32 * a + 32],
                            rhs=xnm_sb[:, t * FW : t * FW + F + 1],
                            start=(i == 0),
                            stop=(i == TL[g] - 1),
                            tile_position=(0, 32 * a),
                        )
                dinv = sbt.tile([128, 1], f32, tag="dinv")
                nc.vector.reciprocal(dinv[:], rps[:, F : F + 1])
                nc.vector.tensor_scalar(r_sb[q][:], rps[:, 0:F], dinv[:], None, ALU.mult)

            # ---------------- steps ----------------
            for s in range(nsteps):
                emit_hT(h_sb, Q1, Q2, halves=(0,))
                ea0, le0 = emit_e(0)
                ean0 = emit_eanT(0, ea0, le0)
                emit_hT(h_sb, Q1, Q2, halves=(1,))
                ea1, le1 = emit_e(1)
                ean1 = emit_eanT(1, ea1, le1)

                emit_attn_tail(0, ean0)
                if s < nsteps - 1:
                    # next LSTM step, half 0: runs while half-1 attention streams
                    emit_hT(r_sb, R1, R2, halves=(0,))
                    lstm_half(0, False)
                else:
                    nc.sync.dma_start(qout_d[0:128, F : 2 * F], r_sb[0][:].bitcast(f32))

                emit_attn_tail(1, ean1)
                if s < nsteps - 1:
                    emit_hT(r_sb, R1, R2, halves=(1,))
                    lstm_half(1, False)
                else:
                    nc.sync.dma_start(qout_d[128:256, F : 2 * F], r_sb[1][:].bitcast(f32))
                if s == nsteps - 2:
                    # that was the final LSTM: h is the output q
                    for half in range(2):
                        nc.sync.dma_start(
                            qout_d[128 * half : 128 * half + 128, 0:F],
                            h_sb[half][:].bitcast(f32),
                        )

            if nsteps == 0:
                for half in range(2):
                    nc.sync.dma_start(
                        qout_d[128 * half : 128 * half + 128, 0:F], h_sb[half][:].bitcast(f32)
                    )

    nc.compile()
    return nc


def _get_program(TL) -> bass.Bass:
    nsteps = int(os.environ.get("KERNEL_NSTEPS", str(STEPS)))
    key = (tuple(TL), nsteps)
    if key not in _PROG_CACHE:
        _PROG_CACHE[key] = _build_program(TL, nsteps)
    return _PROG_CACHE[key]


def _pack_rows(src, kds, offs, width):
    """Stack row-chunks of src into [128, len(kds)*width] fp16."""
    out = np.zeros((128, len(kds) * width), np.float16)
    for ci, (kd, off) in enumerate(zip(kds, offs)):
        out[0:kd, ci * width : (ci + 1) * width] = src[off : off + kd]
    return out


def _bin_pack(counts, TL):
    """Pack 2048 segments into 64 bins (8 cores x 8 blocks), exactly 32 segs
    per bin, bin g (within core) holding <= TL[g]*128 nodes. Same TL profile
    per core. Returns list of 64 segment-id lists, or None on failure."""
    nbins = CORES * BLOCKS
    caps = np.array([TL[g] * 128 for _ in range(CORES) for g in range(BLOCKS)],
                    dtype=np.int64)
    rem = caps.copy()
    slots = np.full(nbins, BS, dtype=np.int64)
    bins = [[] for _ in range(nbins)]
    order = np.argsort(-counts, kind="stable")
    for s in order:
        avail = np.where(slots > 0)[0]
        b = avail[np.argmax(rem[avail])]
        if rem[b] < counts[s]:
            return None
        bins[b].append(int(s))
        rem[b] -= counts[s]
        slots[b] -= 1
    return bins


def make_in_maps(x, batch, cos_coef, q_star, W_ih, W_hh, b_ih, b_hh):
    """Host-side shard + re-layout. Returns (in_maps, TL, bins)."""
    x = np.ascontiguousarray(np.asarray(x, dtype=np.float32))
    batch = np.asarray(batch).astype(np.int64)
    cos = np.asarray(cos_coef, dtype=np.float32)
    qs = np.asarray(q_star, dtype=np.float32)
    W_ih = np.asarray(W_ih, dtype=np.float32)
    W_hh = np.asarray(W_hh, dtype=np.float32)
    bsum = (np.asarray(b_ih, dtype=np.float32) + np.asarray(b_hh, dtype=np.float32))

    counts = np.bincount(batch, minlength=B)
    starts = np.zeros(B + 1, dtype=np.int64)
    starts[1:] = np.cumsum(counts)

    base = int(max(1, -(-counts.reshape(-1, BS).sum(axis=1).max() // 128)))
    lo = base - 1
    profiles = [
        (base,) + (lo,) * 7,
        (base, base) + (lo,) * 6,
        (base,) * 4 + (lo,) * 4,
        (base,) * 8,
        (base + 1,) * 8,
    ]
    bins = None
    for TL in profiles:
        if min(TL) < 1:
            continue
        bins = _bin_pack(counts, TL)
        if bins is not None:
            break
    assert bins is not None
    TL = list(TL)
    NT = sum(TL)
    BO = [0]
    for t in TL:
        BO.append(BO[-1] + t)

    # LSTM weight stacks (fp16), pre-chunked to 128-row slabs
    W_ihT = W_ih.T  # [400, 800]
    W_hhT = W_hh.T  # [200, 800]
    w0 = np.concatenate(
        [W_ihT, bsum[None, :], W_hhT, np.zeros((BS + 1, 800), np.float32)], axis=0
    )  # [634, 800]
    w0s = _pack_rows(w0, W0_KD, [0, 128, 256, 384, 401, 529], 800)
    WcT = W_ihT[:F] + W_hhT          # [200, 800]
    WrT = W_ihT[F:]                  # [200, 800]
    wc = np.concatenate(
        [WcT[0:128], WcT[128:200], np.zeros((BS + 1, 800), np.float32),
         WrT[0:128], WrT[128:200], bsum[None, :]], axis=0
    )  # [434, 800]
    wcs = _pack_rows(wc, WC_KD, [0, 128, 233, 361], 800)

    qc2c = np.zeros((BS + 1, 256), np.float16)
    qc2c[0:BS] = np.tile(100.0 * np.eye(BS, dtype=np.float32), (1, BLOCKS))
    qc2c[BS] = -100.0

    in_maps = []
    for c in range(CORES):
        xf = np.zeros((KAUG, NT * 128), dtype=np.float16)
        cwt = np.zeros((128, NT * BS), dtype=np.float16)
        xp = np.zeros((128, NT * FW), dtype=np.float16)
        qs0t = np.ones((401, 256), dtype=np.float32)
        for g in range(BLOCKS):
            T = TL[g]
            BW = T * 128
            segs = bins[c * BLOCKS + g]
            idx = np.concatenate(
                [np.arange(starts[s], starts[s + 1]) for s in segs]
            )
            cnt = len(idx)
            js = np.concatenate(
                [np.full(int(counts[s]), j, np.int64) for j, s in enumerate(segs)]
            )

            xb = np.zeros((BW, FW), dtype=np.float32)
            xb[:cnt, :F] = x[idx]
            xb[:cnt, F] = 1.0
            xp[:, BO[g] * FW : BO[g + 1] * FW] = (
                xb.reshape(T, 128, FW).transpose(1, 0, 2).reshape(128, T * FW)
            ).astype(np.float16)

            xfb = np.zeros((KAUG, BW), dtype=np.float32)
            xfb[0:F, :cnt] = x[idx].T
            xfb[F + js, np.arange(cnt)] = 1.0
            xfb[F + BS, :] = 1.0
            xf[:, BO[g] * 128 : BO[g + 1] * 128] = xfb.astype(np.float16)

            wb = np.zeros((BW, BS), dtype=np.float32)
            wb[np.arange(cnt), js] = cos[idx]
            cwt[:, BO[g] * BS : BO[g + 1] * BS] = (
                wb.reshape(T, 128, BS).transpose(1, 0, 2).reshape(128, T * BS)
            ).astype(np.float16)

            qs0t[0:400, g * BS : (g + 1) * BS] = qs[segs].T
        qs0s = _pack_rows(qs0t, QS_KD, [0, 128, 256, 384], 256)
        in_maps.append(
            {
                "xf1": np.ascontiguousarray(xf[0:128]),
                "xf2": np.ascontiguousarray(xf[128:KAUG]),
                "cwt": cwt,
                "xp": xp,
                "qs0s": qs0s,
                "w0s": w0s,
                "wcs": wcs,
                "qc2c": qc2c,
                "onesr": np.ones((1, 256), np.float16),
                "idf": np.eye(128, dtype=np.float32),
            }
        )
    return in_maps, TL, bins


def kernel(x, batch, cos_coef, q_star, W_ih, W_hh, b_ih, b_hh):
    global LAST_RESULT
    in_maps, TL, bins = make_in_maps(
        x, batch, cos_coef, q_star, W_ih, W_hh, b_ih, b_hh
    )
    nc = _get_program(TL)
    res = run_bass_kernel_spmd(nc, in_maps, list(range(CORES)), trace=TRACE)
    LAST_RESULT = res
    out = np.zeros((B, 2 * F), dtype=np.float32)
    for c in range(CORES):
        qo = res.results[c]["qout"]
        for g in range(BLOCKS):
            segs = bins[c * BLOCKS + g]
            out[segs] = qo[g * BS : (g + 1) * BS]
    return out


# revision 41
# speedup vs baseline: 1.0585x; 1.0173x over previous
"""Trainium2 Bass kernel for CognitionNetwork (GNN message passing + LSTM attention).

Contract: kernel(**inputs) takes FULL inputs, returns FULL [2048, 400] q_star.
Shards 2048 conversations across 8 NeuronCores (256 segments each, bin-packed);
each block of 32 segments owns TL[g] 128-node tiles (host re-layout).

v4 design (vs v3.1):
  - segments are bin-packed into blocks with a per-core tile profile TL
    (e.g. [17,16,16,16,16,16,16,16]) identical on every core, minimizing the
    total node-tile count NT and therefore PE streaming work; the host
    permutes segments into blocks and un-permutes the output.
  - attention e from block-level matmuls contracting FEATURES with the mask
    folded into 33 aug rows; exp straight out of PSUM into bf16.
  - EA node-major flip via one XBAR DMA-transpose per q-group (sync queue,
    manual RAW/WAR edges since DMA-transpose reads are not dep-tracked).
  - LSTM for step s+1 emitted inside step s's attention tails (per half);
    sigmoid computed as 0.5*tanh(x/2)+0.5 so the scalar engine never swaps
    activation tables (exp/tanh/copy share one set).
"""

import os
from contextlib import ExitStack

import ml_dtypes
import numpy as np

import concourse.bass as bass
import concourse.bacc as bacc
import concourse.tile as tile
from concourse.tile_rust import add_dep_helper
from concourse import mybir
from concourse.bass_utils import run_bass_kernel_spmd

CORES = 8
B = 2048
F = 200
FW = 201              # node-major x tile width: 200 feats + ones col
SEG_PER_CORE = B // CORES   # 256
BS = 32               # segments per block
BLOCKS = SEG_PER_CORE // BS  # 8
STEPS = 3
KAUG = F + BS + 1     # 233 feature rows incl mask aug
K2 = KAUG - 128       # 105 rows in chunk 2

TRACE = bool(int(os.environ.get("KERNEL_TRACE", "0")))
LAST_RESULT = None
_PROG_CACHE = {}

# LSTM weight chunk profiles (kdim per 128-row slab in the host-packed stacks)
W0_KD = [128, 128, 128, 17, 128, 105]   # q_star(400)+bias | h(200)+augzeros(33)
WC_KD = [128, 105, 128, 73]             # q(200)+aug | r(200)+bias
QS_KD = [128, 128, 128, 17]             # q_star0^T(400)+ones row


def _build_program(TL, nsteps: int = STEPS) -> bass.Bass:
    TL = list(TL)
    NT = sum(TL)                 # node tiles per core
    BO = [0]
    for t in TL:
        BO.append(BO[-1] + t)    # tile offset per block
    XFW = NT * 128               # feature-major x width (nodes)
    Tmax = max(TL)
    TQ = [max(TL[0:4]), max(TL[4:8])]   # per-quad max tiles

    nc = bacc.Bacc("TRN2", target_bir_lowering=False, debug=False)
    f32 = mybir.dt.float32
    f32r = mybir.dt.float32r
    f16 = mybir.dt.float16
    bf16 = mybir.dt.bfloat16
    AF = mybir.ActivationFunctionType
    ALU = mybir.AluOpType

    xf1_d = nc.dram_tensor("xf1", [128, XFW], f16, kind="ExternalInput").ap()
    xf2_d = nc.dram_tensor("xf2", [K2, XFW], f16, kind="ExternalInput").ap()
    cwt_d = nc.dram_tensor("cwt", [128, NT * BS], f16, kind="ExternalInput").ap()
    xp_d = nc.dram_tensor("xp", [128, NT * FW], f16, kind="ExternalInput").ap()
    qs0s_d = nc.dram_tensor("qs0s", [128, 4 * 256], f16, kind="ExternalInput").ap()
    w0s_d = nc.dram_tensor("w0s", [128, 6 * 800], f16, kind="ExternalInput").ap()
    wcs_d = nc.dram_tensor("wcs", [128, 4 * 800], f16, kind="ExternalInput").ap()
    qc2c_d = nc.dram_tensor("qc2c", [BS + 1, 256], f16, kind="ExternalInput").ap()
    ones_d = nc.dram_tensor("onesr", [1, 256], f16, kind="ExternalInput").ap()
    idf_d = nc.dram_tensor("idf", [128, 128], f32r, kind="ExternalInput").ap()
    qout_d = nc.dram_tensor("qout", [256, 400], f32, kind="ExternalOutput").ap()

    with tile.TileContext(nc) as tc:
        with ExitStack() as ctx:
            res = ctx.enter_context(tc.tile_pool(name="res", bufs=1))
            state = ctx.enter_context(tc.tile_pool(name="state", bufs=1))
            eap = ctx.enter_context(tc.tile_pool(name="eap", bufs=2))
            xpp = ctx.enter_context(tc.tile_pool(name="xpp", bufs=5))
            eanp = ctx.enter_context(tc.tile_pool(name="eanp", bufs=2))
            sbt = ctx.enter_context(tc.tile_pool(name="sbt", bufs=2))
            psE = ctx.enter_context(tc.tile_pool(name="psE", bufs=3, space="PSUM"))
            psG = ctx.enter_context(tc.tile_pool(name="psG", bufs=2, space="PSUM"))
            psT = ctx.enter_context(tc.tile_pool(name="psT", bufs=2, space="PSUM"))
            psR = ctx.enter_context(tc.tile_pool(name="psR", bufs=1, space="PSUM"))

            # ---------------- resident tiles ----------------
            idf = res.tile([128, 128], f32r)
            nc.scalar.dma_start(idf[:], idf_d[:])

            cwt_sb = res.tile([128, NT * BS], f16)
            xnm_sb = res.tile([128, NT * FW], bf16)
            xf1_sb = res.tile([128, XFW], f16)
            xf2_sb = res.tile([K2, XFW], f16)

            # transposed-input chunks: Q1/Q2 (h^T + mask const), R1/R2 (r^T + ones)
            Q1 = res.tile([128, 256], f16, tag="Q1", name="Q1")
            Q2 = res.tile([K2, 256], f16, tag="Q2", name="Q2")
            R1 = res.tile([128, 256], f16, tag="R1", name="R1")
            R2 = res.tile([73, 256], f16, tag="R2", name="R2")

            # fp32 state masters (seg-major, two 128-partition halves)
            h_sb = [state.tile([128, F], f32r, tag=f"h{i}", name=f"h{i}") for i in range(2)]
            c_sb = [state.tile([128, F], f32, tag=f"c{i}", name=f"c{i}") for i in range(2)]
            r_sb = [state.tile([128, F], f32r, tag=f"r{i}", name=f"r{i}") for i in range(2)]
            for i in range(2):
                nc.vector.memset(c_sb[i][:], 0.0)

            w0s = res.tile([128, 6 * 800], f16, tag="w0s", name="w0s")
            qs0s = res.tile([128, 4 * 256], f16, tag="qs0s", name="qs0s")

            # ---------------- phase 0: h0 = segment_sum(cos * x) ----------------
            # quad-stacked; streams fp16 x (with ones col) per block, casting it
            # into the resident bf16 node-major copy as it goes
            for q in range(2):
                h0ps = psR.tile([128, F], f32, tag="rblk")
                for a in range(4):
                    g = 4 * q + a
                    T = TL[g]
                    eng = nc.scalar if g % 2 else nc.sync
                    eng.dma_start(
                        cwt_sb[:, BO[g] * BS : BO[g + 1] * BS],
                        cwt_d[:, BO[g] * BS : BO[g + 1] * BS],
                    )
                    xpt = xpp.tile([128, Tmax * FW], f16, tag="xp")
                    XW = T * FW
                    XH = XW // 2
                    o0 = BO[g] * FW
                    nc.sync.dma_start(xpt[:, 0:XH], xp_d[:, o0 : o0 + XH])
                    nc.scalar.dma_start(xpt[:, XH:XW], xp_d[:, o0 + XH : o0 + XW])
                    for i in range(T):
                        nc.tensor.matmul(
                            h0ps[32 * a : 32 * a + 32, :],
                            lhsT=cwt_sb[:, (BO[g] + i) * BS : (BO[g] + i + 1) * BS],
                            rhs=xpt[:, i * FW : i * FW + F],
                            start=(i == 0),
                            stop=(i == T - 1),
                            tile_position=(0, 32 * a),
                        )
                    nc.vector.tensor_copy(xnm_sb[:, o0 : o0 + XW], xpt[:, 0:XW])
                nc.vector.tensor_copy(h_sb[q][:], h0ps[:])

            # remaining loads, in consumption order: LSTM0 weights + aug, then
            # feature-major x for attention, then step>=1 weights
            nc.sync.dma_start(w0s[:], w0s_d[:])
            nc.scalar.dma_start(qs0s[:], qs0s_d[:])
            nc.sync.dma_start(Q2[72:K2, :], qc2c_d[:])
            for g in range(BLOCKS):
                eng = nc.scalar if g < 4 else nc.sync
                c0, c1 = BO[g] * 128, BO[g + 1] * 128
                eng.dma_start(xf1_sb[:, c0:c1], xf1_d[:, c0:c1])
                eng.dma_start(xf2_sb[:, c0:c1], xf2_d[:, c0:c1])
            wcs = res.tile([128, 4 * 800], f16, tag="wcs", name="wcs")
            nc.scalar.dma_start(wcs[:], wcs_d[:])
            nc.sync.dma_start(R2[72:73, :], ones_d[:])

            def emit_hT(src_halves, dst1, dst2, halves=(0, 1)):
                """transpose seg-major [128,200] f32r halves into fp16 feat-major
                chunks: dst1[:, co:co+128] rows 0..127, dst2[0:72, ...] rows 128..199."""
                for half in halves:
                    src = src_halves[half]
                    co = 128 * half
                    t1 = psT.tile([128, 128], f32r, tag="tp")
                    nc.tensor.transpose(t1[:], src[:, 0:128], idf[:])
                    nc.vector.tensor_copy(dst1[:, co : co + 128], t1[:].bitcast(f32))
                    t2 = psT.tile([72, 128], f32r, tag="tp")
                    nc.tensor.transpose(t2[:], src[:, 128:200], idf[:])
                    nc.vector.tensor_copy(dst2[0:72, co : co + 128], t2[:].bitcast(f32))

            emit_hT(h_sb, Q1, Q2)

            # ---------------- LSTM cell ----------------
            def lstm_half(half, step0):
                co = 128 * half
                if step0:
                    chunks = [(qs0s, ci * 256, kd) for ci, kd in enumerate(QS_KD)]
                    chunks += [(Q1, None, 128), (Q2, None, K2)]
                    wts = w0s
                else:
                    chunks = [(Q1, None, 128), (Q2, None, K2),
                              (R1, None, 128), (R2, None, 73)]
                    wts = wcs
                # sigmoid(x) = 0.5*tanh(x/2) + 0.5: keeps every scalar-engine
                # activation (exp/tanh/copy) inside one act-table set -> no
                # ACT_TABLE_LOAD swaps on the critical path.
                acts = {}
                for part in range(2):
                    ps = psG.tile([128, 400], f32, tag="gates")
                    nch = len(chunks)
                    for ci, (ctile, coff, kdim) in enumerate(chunks):
                        lhsT = (ctile[0:kdim, coff + co : coff + co + 128]
                                if coff is not None
                                else ctile[0:kdim, co : co + 128])
                        nc.tensor.matmul(
                            ps[:],
                            lhsT=lhsT,
                            rhs=wts[0:kdim, ci * 800 + 400 * part : ci * 800 + 400 * part + 400],
                            start=(ci == 0),
                            stop=(ci == nch - 1),
                        )
                    if part == 0:
                        # both gates take scale=0.5: one batched activation
                        tif = sbt.tile([128, 400], f32, tag="tif")
                        nc.scalar.activation(tif[:], ps[:], AF.Tanh, scale=0.5)
                        nc.vector.tensor_scalar(tif[:], tif[:], 0.5, 0.5, ALU.mult, ALU.add)
                        acts["i"], acts["f"] = tif[:, 0:F], tif[:, F:400]
                    else:
                        tg = sbt.tile([128, F], f32, tag="tg")
                        nc.scalar.activation(tg[:], ps[:, 0:F], AF.Tanh)
                        to = sbt.tile([128, F], f32, tag="so")
                        nc.scalar.activation(to[:], ps[:, F:400], AF.Tanh, scale=0.5)
                        nc.vector.tensor_scalar(to[:], to[:], 0.5, 0.5, ALU.mult, ALU.add)
                        acts["g"], acts["o"] = tg[:], to[:]
                ch = c_sb[half]
                tmp = sbt.tile([128, F], f32, tag="tmp")
                nc.vector.tensor_mul(tmp[:], acts["f"], ch[:])
                nc.vector.tensor_mul(ch[:], acts["i"], acts["g"])
                nc.vector.tensor_add(ch[:], tmp[:], ch[:])
                # tg is dead after the c update: reuse it for tanh(c)
                tct = acts["g"]
                nc.scalar.activation(tct, ch[:], AF.Tanh)
                nc.vector.tensor_mul(h_sb[half][:], acts["o"], tct)

            # first LSTM step (h0 + given q_star)
            if nsteps >= 1:
                lstm_half(0, True)
                lstm_half(1, True)
                if nsteps == 1:
                    for half in range(2):
                        nc.sync.dma_start(
                            qout_d[128 * half : 128 * half + 128, 0:F],
                            h_sb[half][:].bitcast(f32),
                        )

            # ---------------- attention ----------------
            prev_dmat = [None, None]

            def emit_e(q):
                """e_aug matmuls + exp for 4 stacked blocks -> EA [128, TQ[q]*128] bf16."""
                BWq = TQ[q] * 128
                ea = eap.tile([128, Tmax * 128], bf16, tag="ea", name="ea")
                exps = []
                NCH = (BWq + 511) // 512
                for k in range(NCH):
                    c0 = k * 512
                    cwm = min(512, BWq - c0)
                    pe = psE.tile([128, 512], f32, tag="pe")
                    for a in range(4):
                        g = 4 * q + a
                        cw = min(512, TL[g] * 128 - c0)
                        if cw <= 0:
                            continue
                        nb = BO[g] * 128
                        nc.tensor.matmul(
                            pe[32 * a : 32 * a + 32, 0:cw],
                            lhsT=Q1[:, BS * g : BS * (g + 1)],
                            rhs=xf1_sb[:, nb + c0 : nb + c0 + cw],
                            start=True,
                            stop=False,
                            tile_position=(0, 32 * a),
                        )
                        nc.tensor.matmul(
                            pe[32 * a : 32 * a + 32, 0:cw],
                            lhsT=Q2[0:K2, BS * g : BS * (g + 1)],
                            rhs=xf2_sb[0:K2, nb + c0 : nb + c0 + cw],
                            start=False,
                            stop=True,
                            tile_position=(0, 32 * a),
                        )
                    ei = nc.scalar.activation(ea[:, c0 : c0 + cwm], pe[:, 0:cwm], AF.Exp)
                    exps.append(ei)
                if prev_dmat[q] is not None:
                    # the DMA-transpose READ of ea is not dependency-tracked:
                    # order this buffer's first overwrite after the previous
                    # step's transpose explicitly (WAR).
                    add_dep_helper(exps[0].ins, prev_dmat[q].ins,
                                   reason="ea WAR vs untracked dma-transpose read")
                return ea, exps[-1]

            def emit_eanT(q, ea, last_exp):
                """XBAR dma-transpose EA node-major on the sync queue. The
                transpose's READ of ea is not dependency-tracked, so add the
                RAW edge on the last exp writer manually."""
                ean = eanp.tile([128, Tmax * 128], bf16, tag="ean")
                dmat = nc.sync.dma_start(
                    ean[:, 0 : TQ[q] * 128].rearrange("p (t c) -> p t c", t=TQ[q]),
                    ea[:, 0 : TQ[q] * 128],
                    transpose=True,
                )
                add_dep_helper(dmat.ins, last_exp.ins,
                               reason="dma-transpose untracked read of ea (RAW)")
                prev_dmat[q] = dmat
                return ean

            def emit_attn_tail(q, ean):
                """r matmuls over the node-major attention, then normalize."""
                rps = psR.tile([128, F + 1], f32, tag="rblk")
                for i in range(TQ[q]):
                    for a in range(4):
                        g = 4 * q + a
                        if i >= TL[g]:
                            continue
                        t = BO[g] + i
                        nc.tensor.matmul(
                            rps[32 * a : 32 * a + 32, :],
                            lhsT=ean[:, 128 * i + 32 * a : 128 * i + 32 * a + 32],
                            rhs=xnm_sb[:, t * FW : t * FW + F + 1],
                            start=(i == 0),
                            stop=(i == TL[g] - 1),
                            tile_position=(0, 32 * a),
                        )
                dinv = sbt.tile([128, 1], f32, tag="dinv")
                nc.vector.reciprocal(dinv[:], rps[:, F : F + 1])
                nc.vector.tensor_scalar(r_sb[q][:], rps[:, 0:F], dinv[:], None, ALU.mult)

            # ---------------- steps ----------------
            for s in range(nsteps):
                emit_hT(h_sb, Q1, Q2, halves=(0,))
                ea0, le0 = emit_e(0)
                ean0 = emit_eanT(0, ea0, le0)
                emit_hT(h_sb, Q1, Q2, halves=(1,))
                ea1, le1 = emit_e(1)
                ean1 = emit_eanT(1, ea1, le1)

                emit_attn_tail(0, ean0)
                if s < nsteps - 1:
                    # next LSTM step, half 0: runs while half-1 attention streams
                    emit_hT(r_sb, R1, R2, halves=(0,))
                    lstm_half(0, False)
                else:
                    nc.sync.dma_start(qout_d[0:128, F : 2 * F], r_sb[0][:].bitcast(f32))

                emit_attn_tail(1, ean1)
                if s < nsteps - 1:
                    emit_hT(r_sb, R1, R2, halves=(1,))
                    lstm_half(1, False)
                else:
                    nc.sync.dma_start(qout_d[128:256, F : 2 * F], r_sb[1][:].bitcast(f32))
                if s == nsteps - 2:
                    # that was the final LSTM: h is the output q
                    for half in range(2):
                        nc.sync.dma_start(
                            qout_d[128 * half : 128 * half + 128, 0:F],
                            h_sb[half][:].bitcast(f32),
                        )

            if nsteps == 0:
                for half in range(2):
                    nc.sync.dma_start(
                        qout_d[128 * half : 128 * half + 128, 0:F], h_sb[half][:].bitcast(f32)
                    )

    nc.compile()
    return nc


def _get_program(TL) -> bass.Bass:
    nsteps = int(os.environ.get("KERNEL_NSTEPS", str(STEPS)))
    key = (tuple(TL), nsteps)
    if key not in _PROG_CACHE:
        _PROG_CACHE[key] = _build_program(TL, nsteps)
    return _PROG_CACHE[key]


def _pack_rows(src, kds, offs, width):
    """Stack row-chunks of src into [128, len(kds)*width] fp16."""
    out = np.zeros((128, len(kds) * width), np.float16)
    for ci, (kd, off) in enumerate(zip(kds, offs)):
        out[0:kd, ci * width : (ci + 1) * width] = src[off : off + kd]
    return out


def _bin_pack(counts, TL):
    """Pack 2048 segments into 64 bins (8 cores x 8 blocks), exactly 32 segs
    per bin, bin g (within core) holding <= TL[g]*128 nodes. Same TL profile
    per core. Returns list of 64 segment-id lists, or None on failure."""
    nbins = CORES * BLOCKS
    caps = np.array([TL[g] * 128 for _ in range(CORES) for g in range(BLOCKS)],
                    dtype=np.int64)
    rem = caps.copy()
    slots = np.full(nbins, BS, dtype=np.int64)
    bins = [[] for _ in range(nbins)]
    order = np.argsort(-counts, kind="stable")
    for s in order:
        avail = np.where(slots > 0)[0]
        b = avail[np.argmax(rem[avail])]
        if rem[b] < counts[s]:
            return None
        bins[b].append(int(s))
        rem[b] -= counts[s]
        slots[b] -= 1
    return bins


def make_in_maps(x, batch, cos_coef, q_star, W_ih, W_hh, b_ih, b_hh):
    """Host-side shard + re-layout. Returns (in_maps, TL, bins)."""
    x = np.ascontiguousarray(np.asarray(x, dtype=np.float32))
    batch = np.asarray(batch).astype(np.int64)
    cos = np.asarray(cos_coef, dtype=np.float32)
    qs = np.asarray(q_star, dtype=np.float32)
    W_ih = np.asarray(W_ih, dtype=np.float32)
    W_hh = np.asarray(W_hh, dtype=np.float32)
    bsum = (np.asarray(b_ih, dtype=np.float32) + np.asarray(b_hh, dtype=np.float32))

    counts = np.bincount(batch, minlength=B)
    starts = np.zeros(B + 1, dtype=np.int64)
    starts[1:] = np.cumsum(counts)

    base = int(max(1, -(-counts.reshape(-1, BS).sum(axis=1).max() // 128)))
    lo = base - 1
    profiles = [
        (base,) + (lo,) * 7,
        (base, base) + (lo,) * 6,
        (base,) * 4 + (lo,) * 4,
        (base,) * 8,
        (base + 1,) * 8,
    ]
    bins = None
    for TL in profiles:
        if min(TL) < 1:
            continue
        bins = _bin_pack(counts, TL)
        if bins is not None:
            break
    assert bins is not None
    TL = list(TL)
    NT = sum(TL)
    BO = [0]
    for t in TL:
        BO.append(BO[-1] + t)

    # LSTM weight stacks (fp16), pre-chunked to 128-row slabs
    W_ihT = W_ih.T  # [400, 800]
    W_hhT = W_hh.T  # [200, 800]
    w0 = np.concatenate(
        [W_ihT, bsum[None, :], W_hhT, np.zeros((BS + 1, 800), np.float32)], axis=0
    )  # [634, 800]
    w0s = _pack_rows(w0, W0_KD, [0, 128, 256, 384, 401, 529], 800)
    WcT = W_ihT[:F] + W_hhT          # [200, 800]
    WrT = W_ihT[F:]                  # [200, 800]
    wc = np.concatenate(
        [WcT[0:128], WcT[128:200], np.zeros((BS + 1, 800), np.float32),
         WrT[0:128], WrT[128:200], bsum[None, :]], axis=0
    )  # [434, 800]
    wcs = _pack_rows(wc, WC_KD, [0, 128, 233, 361], 800)

    qc2c = np.zeros((BS + 1, 256), np.float16)
    qc2c[0:BS] = np.tile(100.0 * np.eye(BS, dtype=np.float32), (1, BLOCKS))
    qc2c[BS] = -100.0

    in_maps = []
    for c in range(CORES):
        xf = np.zeros((KAUG, NT * 128), dtype=np.float16)
        cwt = np.zeros((128, NT * BS), dtype=np.float16)
        xp = np.zeros((128, NT * FW), dtype=np.float16)
        qs0t = np.ones((401, 256), dtype=np.float32)
        for g in range(BLOCKS):
            T = TL[g]
            BW = T * 128
            segs = bins[c * BLOCKS + g]
            idx = np.concatenate(
                [np.arange(starts[s], starts[s + 1]) for s in segs]
            )
            cnt = len(idx)
            js = np.concatenate(
                [np.full(int(counts[s]), j, np.int64) for j, s in enumerate(segs)]
            )

            xb = np.zeros((BW, FW), dtype=np.float32)
            xb[:cnt, :F] = x[idx]
            xb[:cnt, F] = 1.0
            xp[:, BO[g] * FW : BO[g + 1] * FW] = (
                xb.reshape(T, 128, FW).transpose(1, 0, 2).reshape(128, T * FW)
            ).astype(np.float16)

            xfb = np.zeros((KAUG, BW), dtype=np.float32)
            xfb[0:F, :cnt] = x[idx].T
            xfb[F + js, np.arange(cnt)] = 1.0
            xfb[F + BS, :] = 1.0
            xf[:, BO[g] * 128 : BO[g + 1] * 128] = xfb.astype(np.float16)

            wb = np.zeros((BW, BS), dtype=np.float32)
            wb[np.arange(cnt), js] = cos[idx]
            cwt[:, BO[g] * BS : BO[g + 1] * BS] = (
                wb.reshape(T, 128, BS).transpose(1, 0, 2).reshape(128, T * BS)
            ).astype(np.float16)

            qs0t[0:400, g * BS : (g + 1) * BS] = qs[segs].T
        qs0s = _pack_rows(qs0t, QS_KD, [0, 128, 256, 384], 256)
        in_maps.append(
            {
                "xf1": np.ascontiguousarray(xf[0:128]),
                "xf2": np.ascontiguousarray(xf[128:KAUG]),
                "cwt": cwt,
                "xp": xp,
                "qs0s": qs0s,
                "w0s": w0s,
                "wcs": wcs,
                "qc2c": qc2c,
                "onesr": np.ones((1, 256), np.float16),
                "idf": np.eye(128, dtype=np.float32),
            }
        )
    return in_maps, TL, bins


def kernel(x, batch, cos_coef, q_star, W_ih, W_hh, b_ih, b_hh):
    global LAST_RESULT
    in_maps, TL, bins = make_in_maps(
        x, batch, cos_coef, q_star, W_ih, W_hh, b_ih, b_hh
    )
    nc = _get_program(TL)
    res = run_bass_kernel_spmd(nc, in_maps, list(range(CORES)), trace=TRACE)
    LAST_RESULT = res
    out = np.zeros((B, 2 * F), dtype=np.float32)
    for c in range(CORES):
        qo = res.results[c]["qout"]
        for g in range(BLOCKS):
            segs = bins[c * BLOCKS + g]
            out[segs] = qo[g * BS : (g + 1) * BS]
    return out
